# revision 67
# baseline (speedup 1.0000x reference)
"""Trainium2 Bass kernel for the atomic-descriptor builder (radial Chebyshev +
angular Legendre descriptors, N=256 atoms, minimum-image PBC).

Strategy: shard the central-atom axis i across 8 NeuronCores (32 atoms each).
Per core, pairs live as [128 j-partitions, 64 free cols w = 2*i + c] (chunk
c interleaved innermost so both operands of the ds subtract keep packed
last-dims and hit the DVE 2x fp16 mode).

The O(N^3) triplet sum is reformulated exactly via the monomial expansion of
Legendre polynomials, and the radial Chebyshev ladder is folded into the
host-side linear fold of RAW f32 moments:
  * moving features are [h, x*h, x^2*h, x^3*h] (h = 0.5*fc(cos-half)*mask,
    x = Chebyshev argument); stationary rows are [ones, x^4, x^8] + the 34
    tensor monomials u^alpha (deg 1..4) of the scaled unit vector u = w/b.
  * q_r[k] needs sum_j x^p*h for p=0..8: (row, feature) index pairs
    (ones,f) / (x4,f) / (x8,0) cover p = f, 4+f, 8 -- no T2..T8 ladder.
  * q_ang[n,l] = sum_alpha clp[l,deg]*multinom(alpha)*G[n,alpha]^2 with
    G[n,alpha] = sum_p cheb[n][p]*M[alpha,p] + M[alpha,0], all folded on the
    host in f64 from the raw f32 PSUM moments (better precision than the
    on-device fp16 squaring this replaces).

Scheduling/layout choices (sim-validated against the TRN2 cost model):
  * whole pair pipeline in fp16: TT ops get the DVE 2x mode, tensor_scalar
    ops the 4x mode; the minimum-image wrap w = ds - round(ds) uses the
    fp16 magic-number trick round(ds) = (ds+1536)-1536 as TWO tensor_scalar
    ops (the fused two-scalar form keeps higher internal precision and does
    not round; int16 fixed-point fails because the DVE subtract SATURATES)
  * single 528B/partition input DMA (>=512B avoids the 2x descriptor
    latency multiplier); si is mirrored across the chunk axis on the host
    so the ds AP keeps a packed [1,2] last dim
  * r never materializes: b = sqrt(zc+eps) once on ACT (the only
    table-based activation; a dep-free dummy Sqrt forces the single table
    load into the input-DMA shadow), u = w * recip(b)
  * deg-2/3/4 monomial rows via outer-product APs (stride-0 first dim x
    stride-1 second dim) -- no cyclic duplicate rows, no ext copies
  * cosine cutoff evaluated as cv = cos(pi*sqrt(zc)/2) deg-4 poly in zc on
    the otherwise-idle Pool lanes; fc = cv^2 folded into h
  * 64 matmuls accumulate [37, 32, 4] f32 moments in one PSUM bank; a DVE
    copy stages PSUM->SBUF and the output leaves via an SWDGE scatter-add
    whose 37 identity-indexed 512B descriptors are PREPPED on Pool during
    the compute shadow (outputs are runtime-pre-zeroed so add == write);
    the post-compute trigger_dma skips the HWDGE 625ns + DGE-delay 650ns
    fixed chain. Tile's epilogue drain waits a DMASW-lane semaphore that
    nothing increments for prepare_only descriptors (their completion sem
    is the user's sem=), so _build_program retargets that one wait to the
    descriptor semaphore after scheduling.
  * all remaining math is a host-side f64 fold of the raw f32 moments
"""
import numpy as np
from math import factorial

N_ATOMS = 256
NCORES = 8
NI = N_ATOMS // NCORES        # 32 central atoms per core
NCHUNK = 2                    # j-chunks of 128 partitions
W = NCHUNK * NI               # 64 free columns, w = 2*i + c
NROW = 37                     # stationary rows
NF = 4                        # moving features: h, xh, x2h, x3h
RC = 5.0
LMAX = 4
NA = 4
EPS_B = 1e-6

# fused fp16 input block columns: si (mirrored over c) | sj | mask | idxs
C_SI, C_SJ, C_MASK, C_IDX = 0, 192, 198, 262
NCOL = 266                    # 532B/partition, single 1x DMA

# cos(pi*sqrt(z)/2) Taylor in z (entire function, |err| < 2.6e-5 on [0,1])
_PC = [1.0]
for _k in range(1, 5):
    _PC.append(_PC[-1] * (-(np.pi / 2) ** 2) / ((2 * _k - 1) * (2 * _k)))
PA0, PA1, PA2, PA3, PA4 = [float(v) for v in _PC]

# Chebyshev T_k monomial coefficients, k=0..8 over powers 0..8
CHEB = np.zeros((9, 9))
CHEB[0, 0] = 1.0
CHEB[1, 1] = 1.0
for _k in range(2, 9):
    CHEB[_k, 1:] += 2.0 * CHEB[_k - 1, :-1]
    CHEB[_k, :] -= CHEB[_k - 2, :]

# Legendre P_l coefficients over cos powers 0..4
CLP = np.zeros((LMAX + 1, LMAX + 1))
CLP[0, 0] = 1.0
CLP[1, 1] = 1.0
for _l in range(2, LMAX + 1):
    CLP[_l, 1:] += (2 * _l - 1) / _l * CLP[_l - 1, :-1]
    CLP[_l, :] -= (_l - 1) / _l * CLP[_l - 2, :]

# stationary row table (row index -> monomial alpha); rows 0..2 are
# ones / x^4 / x^8
_U = [(1, 0, 0), (0, 1, 0), (0, 0, 1)]
_D = [(2, 0, 0), (0, 2, 0), (0, 0, 2)]
_R = [(1, 1, 0), (1, 0, 1), (0, 1, 1)]
ALPHAS = [None, None, None] + _U + _D + _R
for _i in range(3):           # P2: row 12+3i+j = u_j * D_i
    for _j in range(3):
        ALPHAS.append(tuple(_U[_j][d] + _D[_i][d] for d in range(3)))
for _i in range(3):           # DR: row 21+3i+j = D_j * R_i
    for _j in range(3):
        ALPHAS.append(tuple(_D[_j][d] + _R[_i][d] for d in range(3)))
ALPHAS.append((1, 1, 1))      # xyz
ALPHAS += [tuple(2 * c for c in a) for a in _D]   # S4: x^4 class
ALPHAS += [tuple(2 * c for c in a) for a in _R]   # SR: x^2y^2 class
assert len(ALPHAS) == NROW


def _multinom(a):
    p = sum(a)
    return factorial(p) // (factorial(a[0]) * factorial(a[1]) * factorial(a[2]))


_compiled = {}


def _build_program(box, debug=False):
    import concourse.bass as bass
    import concourse.bacc as bacc
    import concourse.tile as tile
    from concourse import mybir

    f32 = mybir.dt.float32
    f16 = mybir.dt.float16
    i16 = mybir.dt.int16
    op = mybir.AluOpType
    act = mybir.ActivationFunctionType

    boxf = np.asarray(box, np.float32)
    diag_box = float(np.abs(boxf - np.diag(np.diag(boxf))).max()) == 0.0
    eq_diag = diag_box and boxf[0, 0] == boxf[1, 1] == boxf[2, 2]
    L = float(boxf[0, 0])
    SCL = L if eq_diag else 1.0   # w stays fractional only for eq-diag
    ZSC = float((SCL / RC) ** 2)  # rsq -> zc scale

    nc = bacc.Bacc("TRN2", target_bir_lowering=False, debug=False,
                   enable_asserts=False)

    insd = nc.dram_tensor("ins", [128, NCOL], f16, kind="ExternalInput")
    outd = nc.dram_tensor("outt", [NROW, NI * NF], f32, kind="ExternalOutput")

    def vap(t, r0, pattern, cols=slice(0, W)):
        """AP over tile t rows starting at r0 with row-structured dims.
        pattern = list of (row_step, count); innermost = the col slice."""
        base = t[:, r0, cols]
        rs = t[:, 1, :].offset - t[:, 0, :].offset
        dims = [base.ap[0]] + [[st * rs, n] for st, n in pattern] \
            + [list(base.ap[-1])]
        return bass.AP(tensor=base.tensor, offset=base.offset, ap=dims)

    with tile.TileContext(nc) as tc:
        with tc.tile_pool(name="sb", bufs=1) as sb, \
             tc.tile_pool(name="ps", bufs=1, space="PSUM") as ps, \
             nc.allow_low_precision(reason="fp16 pair pipeline, f32 moments"):

            def t(shape, tag, dt=f16):
                return sb.tile(shape, dt, tag=tag, name=tag)

            ins = t([128, NCOL], "ins")
            nc.sync.dma_start(out=ins[:, :], in_=insd.ap())
            m_cols = ins[:, C_MASK:C_MASK + W]

            dsw = t([128, 3, W], "dsw")
            rnd = t([128, 3, W], "rnd")
            wv = t([128, 3, W], "wv")
            dr2 = t([128, 3, W], "dr2")
            rsq = t([128, W], "rsq")
            b = t([128, W], "b")
            bc = t([128, W], "bc")
            rb = t([128, W], "rb")
            zc = t([128, W], "zc")
            zc21 = t([128, W], "zc21")
            z2 = t([128, W], "z2")
            e0 = t([128, W], "e0")
            e1 = t([128, W], "e1")
            f1 = t([128, W], "f1")
            p_ = t([128, W], "p_")
            cv = t([128, W], "cv")
            hm = t([128, W], "hm")
            XL = t([128, 3, W], "XL")          # x, x^2, x^3
            Tt = t([128, NROW, W], "Tt")
            mov = t([128, NF, W], "mov")
            OT = t([128, 1, NI * NF], "OT", f32)
            b_eps = t([128, 1], "b_eps", f32)

            pm = ps.tile([NROW, NI, NF], mybir.dt.float32, tag="pm",
                         name="pm")

            # ---- constants / table load (input-DMA shadow) --------------
            nc.gpsimd.memset(Tt[:, 0, :], 1.0)
            nc.gpsimd.memset(b_eps, EPS_B)
            nc.gpsimd.memset(OT[:, :, :], 0.0)
            # dep-free first ACT op: forces the single sqrt-set table load
            # to run inside the input-DMA shadow
            nc.scalar.activation(out=f1[:, 0:1], in_=b_eps[:, :],
                                 func=act.Sqrt, bias=b_eps[:, :])

            # scatter-descriptor prep: first Pool op after the input lands
            # (only reads the idx cols; the OT read defers to the trigger)
            idx_ap = ins[0:16, C_IDX:C_IDX + 3].bitcast(i16)
            dma_sem = nc.alloc_semaphore("outsem")
            dma_sem_ref = [dma_sem]
            with tc.high_priority():
                nc.gpsimd.dma_scatter_add(
                    outd.ap(), OT[:, :, :], idx_ap, NROW, NROW, NI * NF,
                    prepare_only=True, sem=dma_sem)

            # ---- distance head (DVE) -----------------------------------
            # si mirrored over c so both last dims stay packed (2x mode)
            HW_ = W // 2
            for h0 in (0, HW_):
                cs = slice(h0, h0 + HW_)
                ds_o = bass.AP(tensor=dsw[:, :, cs].tensor,
                               offset=dsw[:, :, cs].offset,
                               ap=[dsw[:, :, cs].ap[0], [W, 3], [2, NI // 2],
                                   [1, 2]])
                si_v = bass.AP(tensor=ins[:, :].tensor,
                               offset=ins[:, C_SI + h0:C_SI + h0 + 1].offset,
                               ap=[ins[:, :].ap[0], [W, 3], [2, NI // 2],
                                   [1, 2]])
                sj_v = bass.AP(tensor=ins[:, :].tensor,
                               offset=ins[:, C_SJ:C_SJ + 1].offset,
                               ap=[ins[:, :].ap[0], [2, 3], [0, NI // 2],
                                   [1, 2]])
                nc.vector.tensor_tensor(out=ds_o, in0=si_v, in1=sj_v,
                                        op=op.subtract)
            # minimum image via fp16 magic-number round (two 4x TS ops);
            # all spine ops half-width pipelined through the ack windows
            for h0 in (0, HW_):
                cs = slice(h0, h0 + HW_)
                nc.vector.tensor_scalar(out=rnd[:, :, cs],
                                        in0=dsw[:, :, cs],
                                        scalar1=1536.0, scalar2=None,
                                        op0=op.add)
            for h0 in (0, HW_):
                cs = slice(h0, h0 + HW_)
                nc.vector.tensor_scalar(out=rnd[:, :, cs],
                                        in0=rnd[:, :, cs],
                                        scalar1=1536.0, scalar2=None,
                                        op0=op.subtract)
            for h0 in (0, HW_):
                cs = slice(h0, h0 + HW_)
                nc.vector.tensor_tensor(out=wv[:, :, cs],
                                        in0=dsw[:, :, cs],
                                        in1=rnd[:, :, cs], op=op.subtract)
            if not diag_box:
                # general box: dr = B @ w (Cartesian, fp16)
                drt = t([128, 3, W], "drt")
                for d in range(3):
                    nc.vector.tensor_scalar(
                        out=drt[:, d, :], in0=wv[:, 0, :],
                        scalar1=float(boxf[d, 0]), scalar2=None, op0=op.mult)
                    for e in (1, 2):
                        nc.vector.scalar_tensor_tensor(
                            out=drt[:, d, :], in0=wv[:, e, :],
                            scalar=float(boxf[d, e]), in1=drt[:, d, :],
                            op0=op.mult, op1=op.add)
                wv = drt
            elif not eq_diag:
                for d in range(3):
                    nc.vector.tensor_scalar(
                        out=wv[:, d, :], in0=wv[:, d, :],
                        scalar1=float(boxf[d, d]), scalar2=None, op0=op.mult)
            for hs in (slice(0, W // 2), slice(W // 2, W)):
                nc.vector.tensor_tensor(out=dr2[:, :, hs], in0=wv[:, :, hs],
                                        in1=wv[:, :, hs], op=op.mult)
            for hs in (slice(0, W // 2), slice(W // 2, W)):
                nc.vector.tensor_reduce(
                    out=rsq[:, hs],
                    in_=dr2[:, :, hs].rearrange("p d w -> p w d"),
                    axis=mybir.AxisListType.X, op=op.add)

            # ---- b = sqrt(zc+eps) (ACT), reciprocal + units (DVE) ------
            # half-width pipelined so recip/u start on the first half while
            # ACT computes the second
            H0, H1 = slice(0, W // 2), slice(W // 2, W)
            for hs in (H0, H1):
                nc.scalar.activation(out=b[:, hs], in_=rsq[:, hs],
                                     func=act.Sqrt, scale=ZSC,
                                     bias=b_eps[:, :])
            nc.vector.reciprocal(out=rb[:, :], in_=b[:, :])
            nc.vector.tensor_scalar(out=bc[:, :], in0=b[:, :], scalar1=1.0,
                                    scalar2=0.25, op0=op.min,
                                    op1=op.subtract)
            rb_b = bass.AP(tensor=rb[:, :].tensor, offset=rb[:, :].offset,
                           ap=[rb[:, :].ap[0], [0, 3], [1, W]])
            nc.vector.tensor_tensor(out=Tt[:, 3:6, :], in0=wv[:, :, :],
                                    in1=rb_b, op=op.mult)          # u

            # ---- Pool lane: cosine cutoff cv = cos(pi*sqrt(zc)/2) ------
            nc.gpsimd.tensor_scalar(out=zc[:, :], in0=rsq[:, :],
                                    scalar1=ZSC, scalar2=1.0,
                                    op0=op.mult, op1=op.min)
            nc.vector.tensor_scalar(out=zc21[:, :], in0=rsq[:, :],
                                    scalar1=2.0 * ZSC, scalar2=2.0,
                                    op0=op.mult, op1=op.min)
            nc.gpsimd.tensor_tensor(out=z2[:, :], in0=zc[:, :],
                                    in1=zc[:, :], op=op.mult)
            nc.scalar.activation(out=e1[:, :], in_=zc[:, :], func=act.Copy,
                                 scale=PA3, bias=PA2)
            nc.scalar.activation(out=e0[:, :], in_=zc[:, :], func=act.Copy,
                                 scale=PA1, bias=PA0)
            nc.gpsimd.tensor_scalar(out=f1[:, :], in0=z2[:, :],
                                    scalar1=PA4, scalar2=None, op0=op.mult)
            nc.gpsimd.tensor_tensor(out=f1[:, :], in0=f1[:, :],
                                    in1=e1[:, :], op=op.add)
            nc.gpsimd.tensor_tensor(out=p_[:, :], in0=z2[:, :],
                                    in1=f1[:, :], op=op.mult)
            nc.gpsimd.tensor_tensor(out=cv[:, :], in0=p_[:, :],
                                    in1=e0[:, :], op=op.add)
            # beyond rc, zc caps at 1 so cv = cos(pi/2) ~ 0 kills far
            # pairs automatically (residual h ~ cv^2 ~ 1e-9); only the
            # self-exclusion mask from the host is needed
            nc.gpsimd.tensor_tensor(out=hm[:, :], in0=cv[:, :],
                                    in1=m_cols, op=op.mult)
            nc.gpsimd.tensor_tensor(out=mov[:, 0, :], in0=cv[:, :],
                                    in1=hm[:, :], op=op.mult)      # h

            # ---- x cluster (DVE) ---------------------------------------
            nc.vector.scalar_tensor_tensor(out=XL[:, 0, :], in0=bc[:, :],
                                           scalar=-4.0, in1=zc21[:, :],
                                           op0=op.mult, op1=op.add)  # x
            nc.vector.tensor_tensor(out=XL[:, 1, :], in0=XL[:, 0, :],
                                    in1=XL[:, 0, :], op=op.mult)     # x^2
            nc.vector.tensor_tensor(out=XL[:, 2, :], in0=XL[:, 0, :],
                                    in1=XL[:, 1, :], op=op.mult)     # x^3
            nc.vector.tensor_tensor(out=Tt[:, 1, :], in0=XL[:, 1, :],
                                    in1=XL[:, 1, :], op=op.mult)     # x^4
            nc.vector.tensor_tensor(out=Tt[:, 2, :], in0=Tt[:, 1, :],
                                    in1=Tt[:, 1, :], op=op.mult)     # x^8

            # ---- monomial rows (DVE outer products + ACT squares) ------
            nc.scalar.activation(out=Tt[:, 6:9, :], in_=Tt[:, 3:6, :],
                                 func=act.Square)                    # D
            nc.vector.tensor_tensor(out=vap(Tt, 9, [(1, 2)]),
                                    in0=vap(Tt, 3, [(0, 2)]),
                                    in1=vap(Tt, 4, [(1, 2)]),
                                    op=op.mult)                      # xy, xz
            nc.vector.tensor_tensor(out=Tt[:, 11, :], in0=Tt[:, 4, :],
                                    in1=Tt[:, 5, :], op=op.mult)     # yz
            nc.vector.tensor_tensor(out=Tt[:, 30, :], in0=Tt[:, 3, :],
                                    in1=Tt[:, 11, :], op=op.mult)    # xyz
            nc.vector.tensor_tensor(out=Tt[:, 12:21, :],
                                    in0=vap(Tt, 3, [(0, 3), (1, 3)]),
                                    in1=vap(Tt, 6, [(1, 3), (0, 3)]),
                                    op=op.mult)                      # u x D
            nc.vector.tensor_tensor(out=Tt[:, 21:30, :],
                                    in0=vap(Tt, 6, [(0, 3), (1, 3)]),
                                    in1=vap(Tt, 9, [(1, 3), (0, 3)]),
                                    op=op.mult)                      # D x R
            nc.scalar.activation(out=Tt[:, 31:34, :], in_=Tt[:, 6:9, :],
                                 func=act.Square)                    # D^2
            nc.scalar.activation(out=Tt[:, 34:37, :], in_=Tt[:, 9:12, :],
                                 func=act.Square)                    # R^2

            # ---- moving features: h, x*h, x^2*h, x^3*h -----------------
            h_b = bass.AP(tensor=mov[:, 0, :].tensor,
                          offset=mov[:, 0, :].offset,
                          ap=[mov[:, 0, :].ap[0], [0, 3], [1, W]])
            nc.vector.tensor_tensor(out=mov[:, 1:4, :], in0=XL[:, :, :],
                                    in1=h_b, op=op.mult)

            # ---- PE: per-atom moment matmuls ---------------------------
            for i in range(NI):
                for c in range(NCHUNK):
                    col = 2 * i + c
                    nc.tensor.matmul(pm[:, i, :], Tt[:, :, col:col + 1],
                                     mov[:, :, col:col + 1],
                                     start=(c == 0), stop=(c == NCHUNK - 1))

            # ---- stage PSUM -> SBUF + trigger, both on Pool ------------
            # (HBM outputs are pre-zeroed by the runtime, so scatter-ADD of
            # 37 identity-indexed 512B rows == plain write; the trigger
            # skips the HWDGE 625ns + DGE 650ns fixed chain, and sharing
            # the engine with the copy avoids a cross-engine sem hop)
            nc.vector.tensor_copy(out=OT[0:NROW, 0, :], in_=pm[:, :, :])
            nc.gpsimd.trigger_dma(count=None)

            if debug:
                dbgd = nc.dram_tensor("dbg", [128, NROW * W], f16,
                                      kind="ExternalOutput")
                dbg = t([128, NROW, W], "dbg")
                nc.vector.tensor_copy(out=dbg[:, 0:3, :], in_=wv[:, :, :])
                nc.vector.tensor_copy(out=dbg[:, 3, :], in_=rsq[:, :])
                nc.vector.tensor_copy(out=dbg[:, 4, :], in_=b[:, :])
                nc.vector.tensor_copy(out=dbg[:, 5, :], in_=cv[:, :])
                nc.vector.tensor_copy(out=dbg[:, 6:10, :], in_=mov[:, :, :])
                nc.vector.tensor_copy(out=dbg[:, 10:13, :], in_=XL[:, :, :])
                nc.vector.tensor_copy(out=dbg[:, 13:37, :],
                                      in_=Tt[:, 0:24, :])
                nc.sync.dma_start(out=dbgd.ap()[:, :], in_=dbg[:, :, :])

    # The framework preamble serializes four const-AP memsets on Pool ahead
    # of the entry barrier (~380ns); three of the consts (f32-1.0, bf16-1.0,
    # u8-127) are never read by this program (birverifier confirms), so drop
    # their memsets -- verified below by scanning every instruction's inputs.
    import re as _re
    _dead = {"const-float32-1.0", "const-bfloat16-1.0", "const-uint8-127"}
    _read = set()
    for blk in nc.main_func.blocks:
        for insn in blk.instructions:
            for a in insn.ins:
                for m in _re.finditer(r"name='(const-[^']+)'", str(a)):
                    _read.add(m.group(1))
    _dead -= _read
    blk0 = nc.main_func.blocks[0]
    blk0.instructions[:] = [
        insn for insn in blk0.instructions
        if not (type(insn).__name__ == "InstMemset" and insn.outs
                and _re.search(r"name='([^']+)'", str(insn.outs[0]))
                and _re.search(r"name='([^']+)'",
                               str(insn.outs[0])).group(1) in _dead)
    ]

    # Tile's epilogue drain waits the prep's DMASW lane sem, but for
    # prepare_only the descriptor's completion sem is the user's sem= (on
    # hardware SDMA bumps on_update[0] by 16); retarget the orphan wait in
    # our own program IR so sim and HW agree.
    for blk in nc.main_func.blocks:
        for insn in blk.instructions:
            si = insn.sync_info
            if not si:
                continue
            for wt in si.on_wait:
                if wt.ant_name and 'DMASW' in str(wt.ant_name) \
                        and wt.wait_value == 16:
                    wt.id = dma_sem_ref[0].num
                    wt.ant_name = dma_sem_ref[0].name

    nc.compile()
    return nc


def _host_prep(R, box):
    R = np.asarray(R, np.float64)
    boxf = np.asarray(box, np.float64)
    box_inv = np.linalg.inv(boxf)
    s = np.mod(R @ box_inv.T, 1.0)                    # fractional in [0,1)
    si = s.astype(np.float16)
    in_maps = []
    for r in range(NCORES):
        ins = np.zeros((128, NCOL), np.float16)
        sl = si[r * NI:(r + 1) * NI, :]               # [NI,3]
        for d in range(3):
            blk = np.repeat(sl[:, d], 2)              # mirror over c
            ins[:, C_SI + d * W:C_SI + (d + 1) * W] = blk
        for c in range(NCHUNK):
            for d in range(3):
                ins[:, C_SJ + d * 2 + c] = si[c * 128:(c + 1) * 128, d]
        m = np.full((128, W), 0.5, np.float16)        # 0.5*mask (h scale)
        for i in range(NI):
            g = r * NI + i
            c, j = divmod(g, 128)
            m[j, 2 * i + c] = 0.0
        ins[:, C_MASK:C_MASK + W] = m
        # scatter row indices (identity, -1 pad), int16 bits in fp16 cols
        idx = np.full((16, 3), -1, np.int16)
        for k in range(NROW):
            idx[k % 16, k // 16] = k
        ins[0:16, C_IDX:C_IDX + 3] = idx.view(np.float16)
        in_maps.append({"ins": ins})
    return in_maps


def _fold_tables(box):
    """Precompute host fold matrices for a given box."""
    boxf = np.asarray(box, np.float64)
    diag_box = float(np.abs(boxf - np.diag(np.diag(boxf))).max()) == 0.0
    eq_diag = diag_box and boxf[0, 0] == boxf[1, 1] == boxf[2, 2]
    uscale = (float(boxf[0, 0]) / RC) if eq_diag else (1.0 / RC)
    return uscale


def kernel(R, box):
    R = np.asarray(R)
    box = np.asarray(box)
    key = np.asarray(box, np.float32).tobytes()
    nc = _compiled.get(key)
    if nc is None:
        nc = _build_program(box)
        _compiled[key] = nc
    in_maps = _host_prep(R, box)
    from concourse.bass_utils import run_bass_kernel_spmd
    res = run_bass_kernel_spmd(nc, in_maps, core_ids=list(range(NCORES)))

    uscale = _fold_tables(box)
    parts = []
    for r in range(NCORES):
        M = res.results[r]["outt"].astype(np.float64)     # [37, 128]
        M = M.reshape(NROW, NI, NF)
        out = np.zeros((NI, 9 + NA * (LMAX + 1)))
        # q_r from x-power moments
        Mx = np.zeros((9, NI))
        Mx[0:4] = M[0, :, 0:4].T
        Mx[4:8] = M[1, :, 0:4].T
        Mx[8] = M[2, :, 0]
        for k in range(9):
            out[:, k] = CHEB[k, :] @ Mx + Mx[0]
        # q_ang from monomial moments
        for n in range(NA):
            cn = CHEB[n, 0:4]
            g0 = cn @ M[0, :, 0:4].T + M[0, :, 0]         # deg-0 moment
            for l in range(LMAX + 1):
                acc = CLP[l, 0] * g0 ** 2
                for c in range(3, NROW):
                    al = ALPHAS[c]
                    deg = sum(al)
                    if CLP[l, deg] == 0.0:
                        continue
                    G = (cn @ M[c, :, 0:4].T + M[c, :, 0]) * uscale ** deg
                    acc = acc + CLP[l, deg] * _multinom(al) * G ** 2
                out[:, 9 + n * (LMAX + 1) + l] = acc
        parts.append(out)
    return np.concatenate(parts, axis=0).astype(np.float32)
